# revision 1
# baseline (speedup 1.0000x reference)
"""NT-Xent (SimCLR) contrastive loss on 8 Trainium2 NeuronCores.

Strategy (row-sharded similarity matrix):
  Z = concat(z_i, z_j) -> [N=8192, D=256].  Every core receives the full
  z_i / z_j plus its own raw 1024-row slab of Z.  On device, each core
  - normalizes all N rows (DVE square+accum, bit-trick rsqrt), casts to bf16,
  - stages normalized Z through DRAM and reads it back with the DMA xbar
    transpose to get Zt [D, N] (the layout TensorE needs),
  - computes its [1024, 8192] slab of logits = (Z_slab @ Z^T) via 256 bf16
    matmuls accumulating K=256 in PSUM,
  - applies exp(2*logits) on ScalarE with fused per-partition accumulation
    (accum_out) to produce the row sums of exp(sim/T) directly,
  - DMAs out one [128, 8] f32 tile of slab row sums.
  The host then computes loss = mean(log(rowsum - e^2) - pos/T): subtracting
  e^2 removes the masked diagonal (sim_ii == 1 for normalized rows), and the
  positive-pair dot products are an O(N*D) marshaling-side computation.
"""

import math

import numpy as np

import concourse.bacc as bacc
import concourse.bass as bass
import concourse.mybir as mybir
import concourse.tile as tile
from concourse.bass_utils import run_bass_kernel_spmd

B, D = 4096, 256
N = 2 * B                      # 8192 rows of Z
N_CORES = 8
SLAB = N // N_CORES            # 1024 rows per core
TEMPERATURE = 0.5
INV_T = 1.0 / TEMPERATURE      # 2.0

F32 = mybir.dt.float32
BF16 = mybir.dt.bfloat16
I32 = mybir.dt.int32
ALU = mybir.AluOpType
ACT = mybir.ActivationFunctionType

GROUPS = N // SLAB             # 8 groups of 1024 rows (8x 128-row subtiles)
SUBT = SLAB // 128             # 8 subtiles per group
KT = D // 128                  # 2 contraction tiles
CHUNK = 512                    # matmul moving free dim / PSUM bank
CHUNKS = N // CHUNK            # 16 column chunks
JG = 4                         # chunks per PSUM tile ([128, 2048] = 4 banks)
NJG = CHUNKS // JG             # 4 chunk-groups
MT = SLAB // 128               # 8 output row tiles per core

RSQRT_MAGIC = 0x5F3759DF


def _emit_rsqrt(nc, pool, n2, inv, cols):
    """inv = 1/sqrt(n2), elementwise on a [128, cols] f32 tile.

    Quake-style exponent-halving seed + 3 Newton-Raphson steps, all on DVE
    (ScalarE's Rsqrt table is banned for accuracy and Log/Exp would thrash
    the activation table set against the hot-loop Exp)."""
    t_int = pool.tile([128, cols], I32, tag="rsq_i")
    y = pool.tile([128, cols], F32, tag="rsq_y")
    a = pool.tile([128, cols], F32, tag="rsq_a")
    c = pool.tile([128, cols], F32, tag="rsq_c")
    # y0 = bits^-1(MAGIC - bits(n2) >> 1)  (can't mix bitwise+arith in one op)
    nc.vector.tensor_scalar(
        t_int[:], n2.bitcast(I32), 1, None, op0=ALU.logical_shift_right)
    nc.vector.tensor_scalar(
        y.bitcast(I32), t_int[:], -1, RSQRT_MAGIC, op0=ALU.mult, op1=ALU.add)
    for it in range(2):  # y <- y * (1.5 - 0.5*n2*y^2); ~5e-6 max rel err
        nc.vector.scalar_tensor_tensor(
            a[:], y[:], 1.0, y[:], op0=ALU.bypass, op1=ALU.mult)
        nc.vector.scalar_tensor_tensor(
            c[:], a[:], -0.5, n2, op0=ALU.mult, op1=ALU.mult)
        nc.vector.scalar_tensor_tensor(
            inv if it == 1 else y[:], c[:], 1.5, y[:],
            op0=ALU.add, op1=ALU.mult)


def _emit_normalize_group(nc, pools, raw_src_ap, znorm_dram_ap, zt_dst_aps,
                          level=4):
    """Load 1024 raw f32 rows, L2-normalize them, cast to bf16, stage to DRAM
    and xbar-transpose back into the [128, k, 1024] destination slices.
    level (timing experiments): 1=load only, 2=+norms, 3=+store, 4=+transpose."""
    work, small = pools["work"], pools["small"]
    raw = work.tile([128, SUBT, D], F32, tag="raw")
    nc.sync.dma_start(raw[:], raw_src_ap)
    if level < 2:
        return

    sq_dump = work.tile([128, D], F32, tag="sqdump")
    n2 = small.tile([128, SUBT], F32, tag="n2")
    for t in range(SUBT):
        nc.vector.scalar_tensor_tensor(
            sq_dump[:], raw[:, t], 1.0, raw[:, t],
            op0=ALU.bypass, op1=ALU.mult, accum_out=n2[:, t : t + 1],
        )
    inv = small.tile([128, SUBT], F32, tag="inv")
    _emit_rsqrt(nc, small, n2[:], inv[:], SUBT)

    zn = work.tile([128, SUBT, D], BF16, tag="zn")
    for t in range(SUBT):
        nc.vector.tensor_scalar(
            zn[:, t], raw[:, t], inv[:, t : t + 1], None, op0=ALU.mult)
    if level < 3:
        return

    # Store on the SWDGE (Pool) queue: keeps the in-order SP HWDGE queue
    # free for the load stream, so load(g+1) isn't stuck behind store(g).
    nc.gpsimd.dma_start(
        znorm_dram_ap.rearrange("(n p) d -> p n d", p=128), zn[:]
    )
    if level < 4:
        return
    for k in range(KT):
        # [1024 rows, 128 cols] of staged bf16 -> transposed [128, 1024].
        # Issued from ScalarE's HWDGE queue so the xbar transposes don't
        # interleave with (and serialize against) the copy stream on SP.
        nc.scalar.dma_start(
            out=zt_dst_aps[k],
            in_=znorm_dram_ap[:, k * 128 : (k + 1) * 128],
            transpose=True,
        )


def build_program(repeat=1, phase_a=True, main=True, pa_level=4):
    """repeat>1 re-emits the whole computation N times inside one NEFF —
    used only for steady-state timing (axon RPC latency swamps a single
    ~100us execution).  phase_a/main/pa_level toggle pieces for timing
    experiments (outputs are garbage unless everything is on)."""
    nc = bacc.Bacc(
        "TRN2",
        target_bir_lowering=False,
        debug=False,
        num_devices=N_CORES,
    )
    z_i = nc.declare_dram_parameter("z_i", [B, D], F32, isOutput=False)
    z_j = nc.declare_dram_parameter("z_j", [B, D], F32, isOutput=False)
    z_slab = nc.declare_dram_parameter("z_slab", [SLAB, D], F32, isOutput=False)
    rowsums = nc.declare_dram_parameter("rowsums", [128, MT], F32, isOutput=True)

    zi_t = z_i.rearrange("(n p) d -> p n d", p=128)
    zj_t = z_j.rearrange("(n p) d -> p n d", p=128)
    zs_t = z_slab.rearrange("(n p) d -> p n d", p=128)

    with tile.TileContext(nc) as tc:
        with (
            tc.tile_pool(name="work", bufs=2) as work,
            tc.tile_pool(name="small", bufs=2) as small,
            tc.tile_pool(name="zt", bufs=1) as ztp,
            tc.tile_pool(name="dump", bufs=2) as dump,
            tc.tile_pool(name="psum", bufs=2, space="PSUM") as psum_pool,
            tc.tile_pool(name="dram", bufs=1, space="DRAM") as dram,
        ):
            pools = {"work": work, "small": small}

            # Warm the Exp activation table while DMAs run.
            warm = small.tile([128, 1], F32, tag="warm")
            nc.vector.memset(warm[:], 0.0)
            nc.scalar.activation(warm[:], warm[:], ACT.Exp)

            if not phase_a and main:
                # Timing-only variant: allocate+init matmul operands once.
                zts = ztp.tile([128, KT, SLAB], BF16, tag="zts", name="zts")
                ztn = [
                    ztp.tile([128, KT, SLAB], BF16, tag=f"ztn{g}", name=f"ztn{g}")
                    for g in range(GROUPS)
                ]
                nc.vector.memset(zts[:], 0.0)
                for g in range(GROUPS):
                    nc.vector.memset(ztn[g][:], 0.0)

            for _rep in range(repeat):
                if phase_a:
                    # Persistent transposed normalized embeddings.
                    zts = ztp.tile([128, KT, SLAB], BF16, tag="zts", name="zts")
                    ztn = [
                        ztp.tile(
                            [128, KT, SLAB], BF16, tag=f"ztn{g}", name=f"ztn{g}")
                        for g in range(GROUPS)
                    ]
                    # Slab first: the stationary operand gates every matmul.
                    zns_dram = dram.tile(
                        [SLAB, D], BF16, tag="zslab_dram", name="zslab_dram")
                    _emit_normalize_group(
                        nc, pools, zs_t[:, 0:SUBT],
                        zns_dram[:],
                        [zts[:, k, :] for k in range(KT)],
                        level=pa_level,
                    )
                    # Full Z, one 1024-row group at a time.
                    for g in range(GROUPS):
                        src = (
                            zi_t[:, g * SUBT : (g + 1) * SUBT]
                            if g < GROUPS // 2
                            else zj_t[
                                :,
                                (g - GROUPS // 2) * SUBT
                                : (g - GROUPS // 2 + 1) * SUBT,
                            ]
                        )
                        zn_dram = dram.tile(
                            [SLAB, D], BF16, tag=f"zn_dram{g}", name=f"zn_dram{g}")
                        _emit_normalize_group(
                            nc, pools, src, zn_dram[:],
                            [ztn[g][:, k, :] for k in range(KT)],
                            level=pa_level,
                        )

                if not main:
                    continue
                # Main pass: slab x all-columns logits, exp, fused row sums.
                rsparts = small.tile(
                    [128, MT, NJG], F32, tag="rsparts", name="rsparts")
                for jg in range(NJG):
                    for m in range(MT):
                        ps = psum_pool.tile(
                            [128, JG * CHUNK], F32, tag="ps", name="ps")
                        for j in range(JG):
                            cidx = jg * JG + j
                            g, off = divmod(cidx * CHUNK, SLAB)
                            for k in range(KT):
                                nc.tensor.matmul(
                                    ps[:, j * CHUNK : (j + 1) * CHUNK],
                                    zts[:, k, m * 128 : (m + 1) * 128],
                                    ztn[g][:, k, off : off + CHUNK],
                                    start=(k == 0),
                                    stop=(k == KT - 1),
                                )
                        ex = dump.tile(
                            [128, JG * CHUNK], BF16, tag="ex", name="ex")
                        nc.scalar.activation(
                            ex[:], ps[:], ACT.Exp, scale=INV_T,
                            accum_out=rsparts[:, m, jg : jg + 1],
                        )

                rs = small.tile([128, MT], F32, tag="rs", name="rs")
                nc.vector.tensor_reduce(
                    rs[:].rearrange("p (m o) -> p m o", o=1), rsparts[:],
                    axis=mybir.AxisListType.X, op=ALU.add,
                )
                nc.sync.dma_start(rowsums[:], rs[:])
    nc.compile()
    return nc


_PROGRAM = None


def _get_program():
    global _PROGRAM
    if _PROGRAM is None:
        _PROGRAM = build_program()
    return _PROGRAM


def run_device(z_i, z_j, **spmd_kwargs):
    """Run the SPMD kernel; returns ([N] row sums of exp(sim/T), raw results)."""
    nc = _get_program()
    z_all = np.concatenate([z_i, z_j], axis=0)
    in_maps = [
        {
            "z_i": z_i,
            "z_j": z_j,
            "z_slab": np.ascontiguousarray(z_all[c * SLAB : (c + 1) * SLAB]),
        }
        for c in range(N_CORES)
    ]
    out = run_bass_kernel_spmd(nc, in_maps, list(range(N_CORES)), **spmd_kwargs)
    rowsums = np.concatenate(
        [np.asarray(r["rowsums"]).T.reshape(SLAB) for r in out.results]
    )
    return rowsums, out


def finalize(z_i, z_j, rowsums):
    """Host-side O(N) finish: diagonal removal, log, positive-pair term."""
    rs = rowsums.astype(np.float64)
    lse = np.log(rs - math.exp(INV_T))          # drop masked diagonal exp(1/T)
    zi = z_i.astype(np.float64)
    zj = z_j.astype(np.float64)
    zi /= np.linalg.norm(zi, axis=1, keepdims=True)
    zj /= np.linalg.norm(zj, axis=1, keepdims=True)
    pos = np.sum(zi * zj)                       # = 0.5 * sum_r pos_r
    loss = (lse.sum() - 2.0 * pos * INV_T) / N
    return np.asarray(loss, dtype=np.float32)


def kernel(z_i, z_j):
    z_i = np.ascontiguousarray(np.asarray(z_i, dtype=np.float32))
    z_j = np.ascontiguousarray(np.asarray(z_j, dtype=np.float32))
    rowsums, _ = run_device(z_i, z_j)
    return finalize(z_i, z_j, rowsums)


if __name__ == "__main__":
    rng = np.random.default_rng(0)
    a = rng.standard_normal((B, D), dtype=np.float32)
    b = rng.standard_normal((B, D), dtype=np.float32)
    print(kernel(a, b))

